# revision 6
# baseline (speedup 1.0000x reference)
"""Trainium2 Bass kernel for a 2-layer GCN (segment-sum aggregation).

out = softmax( A @ relu(A @ h @ W1 + b1) @ W2 + b2 ),  A = adjacency (+self loops)

Strategy (8 NeuronCores, node/data parallel), v3:
  - Nodes sharded by range: core k owns nodes [k*12500, (k+1)*12500).
  - Tables are [100000, 128] fp16 rows (64 real cols + 64 pad) so one
    256B dma_gather descriptor fetches exactly one node's features and
    idx = src fits int16 via 4 src-range sub-streams (in_ap base offset).
  - Edges routed to the dst-owning core, laid out per (group of 8 dst
    windows, src-range): per-(window,range) slot counts are the exact
    max over cores (SPMD-uniform), each (group,range) run padded to %128.
  - Aggregation in 256-dst *window pairs*: per 128-slot chunk x pair
    segment, one is_equal one-hot [128,256] (DVE) and one TensorE matmul
    aggT[64,256] += feats.T @ onehot (both layers same orientation).
    Self loops are dense adds.
  - Layer 1: x = relu(aggT@W1+b1) then y = x@W2 (40->64 cols) per window;
    y AllGather'd in chunks (overlapped with layer-1 tail) via contiguous
    staging buffers into a local full-y table with the same row layout.
  - Layer 2: same gather/one-hot streams against the y table; the
    [64,256] pair aggregate is PE-transposed per window, + self loop
    (+b2 pre-folded), softmax on chip.
"""

import math
import os
import numpy as np

D = 64          # input feature dim
ROW = 128       # table row (fp16 cols; 256B)
HID = 128
C = 40
CORES = 8
WIN = 128       # dst window
PAIR = 256      # dst window pair (one-hot width)
NRANGE = 4      # src-range sub-streams (int16 idx reach)
GROUP = 8       # dst windows per gather group
MAXIDX = 1024   # idx per dma_gather call
CC_CHUNKS = int(os.environ.get("GCN_CC_CHUNKS", "4"))


# ----------------------------------------------------------------------------
# Host-side routing
# ----------------------------------------------------------------------------

def route_edges(src, dst, n_nodes):
    nloc = n_nodes // CORES
    nw = math.ceil(nloc / WIN)
    npair = nw // 2
    rsz = n_nodes // NRANGE
    src = src.astype(np.int64)
    dst = dst.astype(np.int64)
    core = dst // nloc
    dloc = dst % nloc
    w = dloc // WIN
    r = src // rsz

    cnt = np.zeros((CORES, nw, NRANGE), np.int64)
    for k in range(CORES):
        m = core == k
        cnt[k] = np.bincount(w[m] * NRANGE + r[m],
                             minlength=nw * NRANGE).reshape(nw, NRANGE)
    S = cnt.max(axis=0)                        # [nw, NRANGE] exact sizes

    ngroups = math.ceil(nw / GROUP)
    groups = [list(range(g * GROUP, min((g + 1) * GROUP, nw)))
              for g in range(ngroups)]

    seg_off = np.zeros((nw, NRANGE), np.int64)
    run_off = np.zeros((ngroups, NRANGE), np.int64)
    run_len = np.zeros((ngroups, NRANGE), np.int64)
    pos = 0
    for g, ws in enumerate(groups):
        for rr in range(NRANGE):
            run_off[g, rr] = pos
            for wi in ws:
                seg_off[wi, rr] = pos
                pos += S[wi, rr]
            P = max(WIN, (pos - run_off[g, rr] + WIN - 1) // WIN * WIN)
            pos = run_off[g, rr] + P
            run_len[g, rr] = P
    tot = pos

    # pair segments: per pair pw (=w//2), ordered (r, chunk):
    # (chunkpos, slot lo, slot hi, segcol, range, block)
    seglists = [[] for _ in range(npair)]
    nseg = 0
    for g, ws in enumerate(groups):
        for rr in range(NRANGE):
            for pw in sorted({wi // 2 for wi in ws}):
                w0, w1 = 2 * pw, 2 * pw + 1
                lo = seg_off[w0, rr]
                hi = seg_off[w1, rr] + S[w1, rr]
                for c in range(lo // WIN, (hi + WIN - 1) // WIN):
                    a, b = max(lo, c * WIN), min(hi, (c + 1) * WIN)
                    if b > a:
                        blk = c - run_off[g, rr] // WIN
                        seglists[pw].append((c * WIN, a, b, nseg, rr, blk))
                        nseg += 1

    idx_streams, dst_streams = [], []
    for k in range(CORES):
        m = core == k
        kk = w[m] * NRANGE + r[m]
        order = np.argsort(kk, kind="stable")
        kk_s = kk[order]
        base = np.r_[0, np.cumsum(np.bincount(kk_s, minlength=nw * NRANGE))[:-1]]
        occ = np.arange(len(kk_s)) - base[kk_s]
        pos_k = seg_off.reshape(-1)[kk_s] + occ
        idx = np.zeros(tot, np.int16)
        idx[pos_k] = (src[m][order] - r[m][order] * rsz).astype(np.int16)
        full = np.full(tot, -1000.0, np.float32)
        full[pos_k] = (dloc[m][order] % PAIR).astype(np.float32)
        dcol = np.full((nseg, WIN), -1000.0, np.float32)
        for pw in range(npair):
            for (cpos, a, b, sc, rr, blk) in seglists[pw]:
                dcol[sc, a - cpos:b - cpos] = full[a:b]
        idx_streams.append(idx)
        dst_streams.append(dcol)

    return dict(S=S, seg_off=seg_off, run_off=run_off, run_len=run_len,
                groups=groups, seglists=seglists, nseg=nseg, tot=tot,
                nw=nw, npair=npair, nloc=nloc, rsz=rsz,
                idx=idx_streams, dst=dst_streams)


# ----------------------------------------------------------------------------
# Bass program
# ----------------------------------------------------------------------------

def build_program(n_nodes, rt, cc_chunks=CC_CHUNKS):
    import concourse.mybir as mybir
    import concourse.bacc as bacc
    from concourse import tile
    import contextlib

    f32 = mybir.dt.float32
    f16 = mybir.dt.float16
    i16 = mybir.dt.int16

    nw, npair, nloc, rsz, tot, nseg = (rt["nw"], rt["npair"], rt["nloc"],
                                       rt["rsz"], rt["tot"], rt["nseg"])
    groups, seglists = rt["groups"], rt["seglists"]
    run_off, run_len = rt["run_off"], rt["run_len"]
    ngroups = len(groups)
    nlocp = nw * WIN
    last_rows = nloc - (nw - 1) * WIN

    nc = bacc.Bacc(None, target_bir_lowering=False, debug=False,
                   num_swdge_queues=4)

    h128 = nc.declare_dram_parameter("h128", [n_nodes, ROW], f16, False)
    hTo = nc.declare_dram_parameter("hTo", [D, nlocp], f16, False)
    W1d = nc.declare_dram_parameter("W1", [D, HID], f32, False)
    b1d = nc.declare_dram_parameter("b1", [HID, 1], f32, False)
    W2d = nc.declare_dram_parameter("W2p", [HID, D], f32, False)
    b2d = nc.declare_dram_parameter("b2b", [WIN, D], f32, False)
    idxd = nc.declare_dram_parameter("idx", [128, tot // 16], i16, False)
    dstd = nc.declare_dram_parameter("dstc", [WIN, nseg], f32, False)
    iotad = nc.declare_dram_parameter("iota2", [WIN, PAIR], f16, False)
    iotacd = nc.declare_dram_parameter("iotac", [WIN, 1], f32, False)
    outd = nc.declare_dram_parameter("out", [nloc, C], f32, True)

    cc_in = nc.dram_tensor("cc_in", [nloc, ROW], f16)
    y4 = nc.dram_tensor("y4", [CORES * nloc, ROW], f16)

    # cc chunk boundaries (group indices) and staging tensors
    gcuts = []
    acc = 0
    for c in range(cc_chunks):
        acc += ngroups // cc_chunks + (1 if c < ngroups % cc_chunks else 0)
        gcuts.append(acc)
    crows = []
    for c in range(cc_chunks):
        w0 = 0 if c == 0 else groups[gcuts[c - 1]][0]
        wend = groups[gcuts[c] - 1][-1] + 1
        r0 = w0 * WIN
        r1 = nloc if wend == nw else wend * WIN
        crows.append((r0, r1))
    ystage = [nc.dram_tensor(f"yst{c}", [CORES * (r1 - r0), ROW], f16,
                             addr_space="Shared")
              for c, (r0, r1) in enumerate(crows)]

    Relu = mybir.ActivationFunctionType.Relu
    Exp = mybir.ActivationFunctionType.Exp
    add_op = mybir.AluOpType.add
    eq_op = mybir.AluOpType.is_equal

    with tile.TileContext(nc) as tc:
        with contextlib.ExitStack() as ctx:
            cpool = ctx.enter_context(tc.tile_pool(name="const", bufs=1))
            ypool = ctx.enter_context(tc.tile_pool(name="yown", bufs=1))

            idx_sb = cpool.tile([128, tot // 16], i16)
            dst_sb = cpool.tile([WIN, nseg], f32)
            iota_sb = cpool.tile([WIN, PAIR], f16)
            iotac_sb = cpool.tile([WIN, 1], f32)
            ident = cpool.tile([D, D], f32)
            hTo_sb = cpool.tile([D, nlocp], f16)
            W1_sb = cpool.tile([D, HID], f32)
            b1_sb = cpool.tile([HID, 1], f32)
            W2_sb = cpool.tile([HID, D], f32)
            b2_sb = cpool.tile([WIN, D], f32)
            yown = ypool.tile([WIN, nw * D], f32)

            nc.sync.dma_start(idx_sb[:], idxd[:])
            nc.sync.dma_start(dst_sb[:], dstd[:])
            nc.sync.dma_start(iota_sb[:], iotad[:])
            nc.sync.dma_start(iotac_sb[:], iotacd[:])
            nc.sync.dma_start(hTo_sb[:], hTo[:])
            nc.sync.dma_start(W1_sb[:], W1d[:])
            nc.sync.dma_start(b1_sb[:], b1d[:])
            nc.sync.dma_start(W2_sb[:], W2d[:])
            nc.sync.dma_start(b2_sb[:], b2d[:])
            # 64x64 identity for PE transposes in layer 2
            nc.vector.tensor_scalar(ident[:], iota_sb[:D, :D],
                                    iotac_sb[:D, 0:1], None, eq_op)

            qctr = [0]

            def issue_gathers(g, gts, table):
                for rr in range(NRANGE):
                    P = int(run_len[g][rr])
                    base = int(run_off[g][rr])
                    gt = gts[rr]
                    for s0 in range(0, P, MAXIDX):
                        ni = min(MAXIDX, P - s0)
                        nblk = (ni + WIN - 1) // WIN
                        nc.gpsimd.dma_gather(
                            out_ap=gt[:, s0:s0 + nblk * WIN]
                            .rearrange("p (c f) -> p c f", f=ROW),
                            in_ap=table[rr * rsz:(rr + 1) * rsz, :],
                            idxs_ap=idx_sb[:, (base + s0) // 16:
                                           (base + s0 + ni) // 16],
                            num_idxs=ni,
                            num_idxs_reg=ni,
                            elem_size=ROW,
                            queue_num=qctr[0] % 4,
                        )
                        qctr[0] += 1

            def make_gts(pool, g, tag):
                return [pool.tile([WIN, (int(run_len[g][rr]) // WIN) * ROW],
                                  f16, tag=f"{tag}{rr}", name=f"{tag}{rr}_{g}")
                        for rr in range(NRANGE)]

            def seg_matmul(ps, gts, pw):
                segs = seglists[pw]
                for i, (cpos, a, b, sc, rr, blk) in enumerate(segs):
                    oh = ohpool.tile([WIN, PAIR], f16, tag="oh", name="oh")
                    nc.vector.tensor_scalar(
                        oh[:], iota_sb[:], dst_sb[:, sc:sc + 1], None, eq_op)
                    feats = gts[rr][:, blk * ROW:blk * ROW + D]
                    nc.tensor.matmul(ps[:], feats, oh[:],
                                     start=(i == 0), stop=(i == len(segs) - 1))

            # ---------------- stage A: layer 1 ----------------
            with contextlib.ExitStack() as sa:
                gpool = sa.enter_context(tc.tile_pool(name="gatherA", bufs=2))
                ohpool = sa.enter_context(tc.tile_pool(name="ohA", bufs=12))
                aggpool = sa.enter_context(tc.tile_pool(name="aggT", bufs=3))
                xpool = sa.enter_context(tc.tile_pool(name="x1", bufs=4))
                psA = sa.enter_context(
                    tc.tile_pool(name="psA", bufs=4, space="PSUM"))
                psB = sa.enter_context(
                    tc.tile_pool(name="psB", bufs=2, space="PSUM"))
                psC = sa.enter_context(
                    tc.tile_pool(name="psC", bufs=2, space="PSUM"))

                cchunk = 0
                for g in range(ngroups):
                    gts = make_gts(gpool, g, "ga")
                    issue_gathers(g, gts, h128)
                    for pw in sorted({wi // 2 for wi in groups[g]}):
                        ps = psA.tile([D, PAIR], f32)
                        seg_matmul(ps, gts, pw)
                        aggT = aggpool.tile([D, PAIR], f32)
                        nc.vector.tensor_tensor(
                            aggT[:], ps[:],
                            hTo_sb[:, pw * PAIR:(pw + 1) * PAIR], add_op)
                        for half in range(2):
                            wi = 2 * pw + half
                            ps2 = psB.tile([HID, WIN], f32)
                            nc.tensor.matmul(
                                ps2[:], W1_sb[:],
                                aggT[:, half * WIN:(half + 1) * WIN],
                                start=True, stop=True)
                            x1 = xpool.tile([HID, WIN], f32)
                            nc.scalar.activation(x1[:], ps2[:], Relu,
                                                 bias=b1_sb[:, 0:1])
                            ps3 = psC.tile([WIN, D], f32)
                            nc.tensor.matmul(ps3[:], x1[:], W2_sb[:],
                                             start=True, stop=True)
                            # yown = y + b2 (self-loop + bias for layer 2)
                            nc.vector.tensor_tensor(
                                yown[:, wi * D:(wi + 1) * D], ps3[:],
                                b2_sb[:], add_op)
                            ybf = xpool.tile([WIN, ROW], f16, tag="ybf")
                            nc.scalar.copy(ybf[:, 0:D], ps3[:])
                            rows = last_rows if wi == nw - 1 else WIN
                            nc.sync.dma_start(
                                cc_in[wi * WIN: wi * WIN + rows, :],
                                ybf[:rows, :])
                    # fire AllGather chunk when its groups are done
                    if cchunk < cc_chunks and g + 1 == gcuts[cchunk]:
                        r0, r1 = crows[cchunk]
                        nc.gpsimd.collective_compute(
                            "AllGather", mybir.AluOpType.bypass,
                            replica_groups=[list(range(CORES))],
                            ins=[cc_in.ap()[r0:r1, :].opt()],
                            outs=[ystage[cchunk].ap().opt()])
                        nst = r1 - r0
                        for k in range(CORES):
                            nc.sync.dma_start(
                                y4.ap()[k * nloc + r0:k * nloc + r1, :],
                                ystage[cchunk].ap()[k * nst:(k + 1) * nst, :])
                        cchunk += 1

            # ---------------- stage C: layer 2 ----------------
            with contextlib.ExitStack() as sc:
                gpool = sc.enter_context(tc.tile_pool(name="gatherC", bufs=2))
                ohpool = sc.enter_context(tc.tile_pool(name="ohC", bufs=12))
                spool = sc.enter_context(tc.tile_pool(name="smax", bufs=6))
                opool = sc.enter_context(tc.tile_pool(name="outp", bufs=4))
                psD = sc.enter_context(
                    tc.tile_pool(name="psD", bufs=4, space="PSUM"))
                psT = sc.enter_context(
                    tc.tile_pool(name="psT", bufs=4, space="PSUM"))

                for g in range(ngroups):
                    gts = make_gts(gpool, g, "gc")
                    issue_gathers(g, gts, y4)
                    for pw in sorted({wi // 2 for wi in groups[g]}):
                        ps = psD.tile([D, PAIR], f32)
                        seg_matmul(ps, gts, pw)
                        tsb = spool.tile([D, PAIR], f32, tag="tsb")
                        nc.scalar.copy(tsb[:], ps[:])
                        for half in range(2):
                            wi = 2 * pw + half
                            pt = psT.tile([WIN, D], f32)
                            nc.tensor.matmul(
                                pt[:], tsb[:, half * WIN:(half + 1) * WIN],
                                ident[:], start=True, stop=True,
                                is_transpose=True)
                            t2 = spool.tile([WIN, D], f32, tag="t2")
                            nc.vector.tensor_tensor(
                                t2[:], pt[:], yown[:, wi * D:(wi + 1) * D],
                                add_op)
                            mx = spool.tile([WIN, 1], f32, tag="mx")
                            nc.vector.tensor_reduce(
                                mx[:], t2[:, :C], mybir.AxisListType.X,
                                mybir.AluOpType.max, negate=True)
                            e = spool.tile([WIN, C], f32, tag="e")
                            nc.scalar.activation(e[:], t2[:, :C], Exp,
                                                 bias=mx[:, 0:1])
                            sm = spool.tile([WIN, 1], f32, tag="sm")
                            nc.vector.tensor_reduce(
                                sm[:], e[:], mybir.AxisListType.X, add_op)
                            ri = spool.tile([WIN, 1], f32, tag="ri")
                            nc.vector.reciprocal(ri[:], sm[:])
                            o = opool.tile([WIN, C], f32)
                            nc.vector.tensor_scalar_mul(o[:], e[:], ri[:, 0:1])
                            rows = last_rows if wi == nw - 1 else WIN
                            nc.sync.dma_start(
                                outd[wi * WIN: wi * WIN + rows, :],
                                o[:rows, :])

    nc.finalize()
    return nc


# ----------------------------------------------------------------------------
# Entry point
# ----------------------------------------------------------------------------

def _prepare_inputs(node_embeddings, adjacency_lists, W1, b1, W2, b2, rt):
    n, d = node_embeddings.shape
    nloc, nw = rt["nloc"], rt["nw"]
    nlocp = nw * WIN
    h = np.ascontiguousarray(node_embeddings, np.float32)
    h128 = np.zeros((n, ROW), np.float16)
    h128[:, :d] = h.astype(np.float16)
    W2p = np.zeros((HID, D), np.float32)
    W2p[:, :C] = W2
    b2b = np.tile(np.pad(b2.astype(np.float32), (0, D - C)), (WIN, 1))
    iota2 = np.tile(np.arange(PAIR, dtype=np.float32), (WIN, 1))
    in_maps = []
    for k in range(CORES):
        hToa = np.zeros((d, nlocp), np.float16)
        hToa[:, :nloc] = h[k * nloc:(k + 1) * nloc].T.astype(np.float16)
        in_maps.append({
            "h128": h128,
            "hTo": hToa,
            "W1": np.ascontiguousarray(W1, np.float32),
            "b1": np.ascontiguousarray(b1, np.float32).reshape(HID, 1),
            "W2p": W2p,
            "b2b": b2b,
            "idx": np.tile(rt["idx"][k].reshape(-1, 16).T, (8, 1)).copy(),
            "dstc": np.ascontiguousarray(rt["dst"][k].T),
            "iota2": iota2.astype(np.float16),
            "iotac": np.arange(WIN, dtype=np.float32).reshape(WIN, 1),
            "out": np.zeros((nloc, C), np.float32),
        })
    return in_maps


_CACHE = {}


def _get_program(n_nodes, rt_sig, rt):
    key = (n_nodes, rt_sig)
    if key not in _CACHE:
        _CACHE[key] = build_program(n_nodes, rt)
    return _CACHE[key]


def kernel(node_embeddings, adjacency_lists, W1, b1, W2, b2, trace=False):
    import sys
    if "/opt/trn_rl_repo" not in sys.path:
        sys.path.insert(0, "/opt/trn_rl_repo")
    from concourse import bass_utils

    n = node_embeddings.shape[0]
    src = np.asarray(adjacency_lists)[:, 0]
    dst = np.asarray(adjacency_lists)[:, 1]
    rt = route_edges(src, dst, n)
    rt_sig = (rt["tot"], rt["nseg"])
    nc = _get_program(n, rt_sig, rt)
    in_maps = _prepare_inputs(node_embeddings, adjacency_lists,
                              W1, b1, W2, b2, rt)
    res = bass_utils.run_bass_kernel_spmd(
        nc, in_maps, core_ids=list(range(CORES)), trace=trace)
    out = np.concatenate([res.results[k]["out"] for k in range(CORES)], axis=0)
    kernel.last_result = res
    return out


# revision 8
# speedup vs baseline: 1.5983x; 1.5983x over previous
"""Trainium2 Bass kernel for a 2-layer GCN (segment-sum aggregation).

out = softmax( A @ relu(A @ h @ W1 + b1) @ W2 + b2 ),  A = adjacency (+self loops)

Strategy (8 NeuronCores, node/data parallel), v4:
  - Nodes sharded by range: core k owns nodes [k*12500, (k+1)*12500).
  - Tables are [100000, 128] fp16 rows (64 real cols + 64 pad) so one
    256B dma_gather descriptor fetches exactly one node's features and
    idx = src fits int16 via 4 src-range sub-streams (in_ap base offset).
  - Edges routed to the dst-owning core, laid out per (group of 8 dst
    windows, src-range): per-(window,range) slot counts are the exact
    max over cores (SPMD-uniform), each (group,range) run padded to %128.
  - Aggregation per 128-dst window: one-hot [128,128] per (chunk x
    window) segment, built in per-(window,range) BATCHES: one DVE
    tensor_tensor is_equal with a stride-0 broadcast of the dst columns
    builds up to MAXB chunk one-hots per instruction. TensorE
    accumulates per-window PSUM. Self loops are dense adds.
  - Layer 1 transforms before layer 2: y = relu(agg@W1+b1)@W2 (40->64
    padded cols); y AllGather'd in chunks (overlapped with layer-1 tail)
    via contiguous staging buffers into a local full-y table with the
    same row layout; layer 2 re-runs the same gather/one-hot streams
    against it with flipped matmul orientation, + self loop (+b2
    pre-folded), softmax on chip (exp row-sums via activation accum).
"""

import math
import os
import numpy as np

D = 64          # input feature dim
ROW = 128       # table row (fp16 cols; 256B)
HID = 128
C = 40
CORES = 8
WIN = 128       # dst window
NRANGE = 4      # src-range sub-streams (int16 idx reach)
GROUP = 8       # dst windows per gather group
MAXIDX = 1024   # idx per dma_gather call
CC_CHUNKS = int(os.environ.get("GCN_CC_CHUNKS", "4"))


# ----------------------------------------------------------------------------
# Host-side routing
# ----------------------------------------------------------------------------

def route_edges(src, dst, n_nodes):
    nloc = n_nodes // CORES
    nw = math.ceil(nloc / WIN)
    rsz = n_nodes // NRANGE
    src = src.astype(np.int64)
    dst = dst.astype(np.int64)
    core = dst // nloc
    dloc = dst % nloc
    w = dloc // WIN
    r = src // rsz

    cnt = np.zeros((CORES, nw, NRANGE), np.int64)
    for k in range(CORES):
        m = core == k
        cnt[k] = np.bincount(w[m] * NRANGE + r[m],
                             minlength=nw * NRANGE).reshape(nw, NRANGE)
    S = cnt.max(axis=0)                        # [nw, NRANGE] exact sizes

    ngroups = math.ceil(nw / GROUP)
    groups = [list(range(g * GROUP, min((g + 1) * GROUP, nw)))
              for g in range(ngroups)]

    seg_off = np.zeros((nw, NRANGE), np.int64)
    run_off = np.zeros((ngroups, NRANGE), np.int64)
    run_len = np.zeros((ngroups, NRANGE), np.int64)
    pos = 0
    for g, ws in enumerate(groups):
        for rr in range(NRANGE):
            run_off[g, rr] = pos
            for wi in ws:
                seg_off[wi, rr] = pos
                pos += S[wi, rr]
            P = max(WIN, (pos - run_off[g, rr] + WIN - 1) // WIN * WIN)
            pos = run_off[g, rr] + P
            run_len[g, rr] = P
    tot = pos

    # batches: per window, per range: consecutive chunks of that
    # (window,range) cell -> one one-hot build, segments share it.
    # batch = (sc0, [(cpos, rr, blk, j), ...]) ; segcol sc = sc0 + j
    batches = [[] for _ in range(nw)]
    nseg = 0
    maxb = 1
    for g, ws in enumerate(groups):
        for rr in range(NRANGE):
            for wi in ws:
                lo, hi = seg_off[wi, rr], seg_off[wi, rr] + S[wi, rr]
                segs = []
                for j, c in enumerate(range(lo // WIN, (hi + WIN - 1) // WIN)):
                    blk = c - run_off[g, rr] // WIN
                    segs.append((c * WIN, max(lo, c * WIN),
                                 min(hi, (c + 1) * WIN), rr, blk, j))
                if segs:
                    batches[wi].append((nseg, segs))
                    nseg += len(segs)
                    maxb = max(maxb, len(segs))

    idx_streams, dst_streams = [], []
    for k in range(CORES):
        m = core == k
        kk = w[m] * NRANGE + r[m]
        order = np.argsort(kk, kind="stable")
        kk_s = kk[order]
        base = np.r_[0, np.cumsum(np.bincount(kk_s, minlength=nw * NRANGE))[:-1]]
        occ = np.arange(len(kk_s)) - base[kk_s]
        pos_k = seg_off.reshape(-1)[kk_s] + occ
        idx = np.zeros(tot, np.int16)
        idx[pos_k] = (src[m][order] - r[m][order] * rsz).astype(np.int16)
        full = np.full(tot, -1000.0, np.float32)
        full[pos_k] = (dloc[m][order] % WIN).astype(np.float32)
        dcol = np.full((nseg, WIN), -1000.0, np.float32)
        for wi in range(nw):
            for (sc0, segs) in batches[wi]:
                for (cpos, a, b, rr, blk, j) in segs:
                    dcol[sc0 + j, a - cpos:b - cpos] = full[a:b]
        idx_streams.append(idx)
        dst_streams.append(dcol)

    return dict(S=S, run_off=run_off, run_len=run_len,
                groups=groups, batches=batches, nseg=nseg, maxb=maxb,
                tot=tot, nw=nw, nloc=nloc, rsz=rsz,
                idx=idx_streams, dst=dst_streams)


# ----------------------------------------------------------------------------
# Bass program
# ----------------------------------------------------------------------------

def build_program(n_nodes, rt, cc_chunks=CC_CHUNKS):
    import concourse.mybir as mybir
    import concourse.bacc as bacc
    from concourse import tile
    import contextlib

    f32 = mybir.dt.float32
    f16 = mybir.dt.float16
    i16 = mybir.dt.int16

    nw, nloc, rsz, tot, nseg, maxb = (rt["nw"], rt["nloc"], rt["rsz"],
                                      rt["tot"], rt["nseg"], rt["maxb"])
    groups, batches = rt["groups"], rt["batches"]
    run_off, run_len = rt["run_off"], rt["run_len"]
    ngroups = len(groups)
    nlocp = nw * WIN
    last_rows = nloc - (nw - 1) * WIN

    nc = bacc.Bacc(None, target_bir_lowering=False, debug=False,
                   num_swdge_queues=4)

    h128 = nc.declare_dram_parameter("h128", [n_nodes, ROW], f16, False)
    hTo = nc.declare_dram_parameter("hTo", [D, nlocp], f16, False)
    W1d = nc.declare_dram_parameter("W1", [D, HID], f32, False)
    b1d = nc.declare_dram_parameter("b1", [HID, 1], f32, False)
    W2d = nc.declare_dram_parameter("W2p", [HID, D], f32, False)
    b2d = nc.declare_dram_parameter("b2b", [WIN, D], f32, False)
    idxd = nc.declare_dram_parameter("idx", [128, tot // 16], i16, False)
    dstd = nc.declare_dram_parameter("dstc", [WIN, nseg], f16, False)
    iotad = nc.declare_dram_parameter("iotaB", [WIN, maxb * WIN], f16, False)
    outd = nc.declare_dram_parameter("out", [nloc, C], f32, True)

    cc_in = nc.dram_tensor("cc_in", [nloc, ROW], f16)
    y4 = nc.dram_tensor("y4", [CORES * nloc, ROW], f16)

    gcuts = []
    acc = 0
    for c in range(cc_chunks):
        acc += ngroups // cc_chunks + (1 if c < ngroups % cc_chunks else 0)
        gcuts.append(acc)
    crows = []
    for c in range(cc_chunks):
        w0 = 0 if c == 0 else groups[gcuts[c - 1]][0]
        wend = groups[gcuts[c] - 1][-1] + 1
        r0 = w0 * WIN
        r1 = nloc if wend == nw else wend * WIN
        crows.append((r0, r1))
    ystage = [nc.dram_tensor(f"yst{c}", [CORES * (r1 - r0), ROW], f16,
                             addr_space="Shared")
              for c, (r0, r1) in enumerate(crows)]

    Relu = mybir.ActivationFunctionType.Relu
    Exp = mybir.ActivationFunctionType.Exp
    add_op = mybir.AluOpType.add
    eq_op = mybir.AluOpType.is_equal

    with tile.TileContext(nc) as tc:
        with contextlib.ExitStack() as ctx:
            cpool = ctx.enter_context(tc.tile_pool(name="const", bufs=1))
            ypool = ctx.enter_context(tc.tile_pool(name="yown", bufs=1))

            idx_sb = cpool.tile([128, tot // 16], i16)
            dst_sb = cpool.tile([WIN, nseg], f16)
            iota_sb = cpool.tile([WIN, maxb * WIN], f16)
            hTo_sb = cpool.tile([D, nlocp], f16)
            W1_sb = cpool.tile([D, HID], f32)
            b1_sb = cpool.tile([HID, 1], f32)
            W2_sb = cpool.tile([HID, D], f32)
            b2_sb = cpool.tile([WIN, D], f32)
            yown = ypool.tile([WIN, nw * D], f32)

            nc.sync.dma_start(idx_sb[:], idxd[:])
            nc.sync.dma_start(dst_sb[:], dstd[:])
            nc.sync.dma_start(iota_sb[:], iotad[:])
            nc.sync.dma_start(hTo_sb[:], hTo[:])
            nc.sync.dma_start(W1_sb[:], W1d[:])
            nc.sync.dma_start(b1_sb[:], b1d[:])
            nc.sync.dma_start(W2_sb[:], W2d[:])
            nc.sync.dma_start(b2_sb[:], b2d[:])

            qctr = [0]

            def issue_gathers(g, gts, table):
                for rr in range(NRANGE):
                    P = int(run_len[g][rr])
                    base = int(run_off[g][rr])
                    gt = gts[rr]
                    for s0 in range(0, P, MAXIDX):
                        ni = min(MAXIDX, P - s0)
                        nblk = (ni + WIN - 1) // WIN
                        nc.gpsimd.dma_gather(
                            out_ap=gt[:, s0:s0 + nblk * WIN]
                            .rearrange("p (c f) -> p c f", f=ROW),
                            in_ap=table[rr * rsz:(rr + 1) * rsz, :],
                            idxs_ap=idx_sb[:, (base + s0) // 16:
                                           (base + s0 + ni) // 16],
                            num_idxs=ni,
                            num_idxs_reg=ni,
                            elem_size=ROW,
                            queue_num=qctr[0] % 4,
                        )
                        qctr[0] += 1

            def make_gts(pool, g, tag):
                return [pool.tile([WIN, (int(run_len[g][rr]) // WIN) * ROW],
                                  f16, tag=f"{tag}{rr}", name=f"{tag}{rr}_{g}")
                        for rr in range(NRANGE)]

            def seg_matmul(ps, gts, wi, layer1):
                bts = batches[wi]
                nb = sum(len(segs) for _, segs in bts)
                i = 0
                for (sc0, segs) in bts:
                    B = len(segs)
                    oh = ohpool.tile([WIN, maxb * WIN], f16,
                                     tag="oh", name="oh")
                    nc.vector.tensor_tensor(
                        oh[:, :B * WIN].rearrange("p (b f) -> p b f", f=WIN),
                        iota_sb[:, :B * WIN].rearrange("p (b f) -> p b f",
                                                       f=WIN),
                        dst_sb[:, sc0:sc0 + B].broadcast_to([WIN, B, WIN]),
                        eq_op)
                    for (cpos, a, b, rr, blk, j) in segs:
                        feats = gts[rr][:, blk * ROW:blk * ROW + D]
                        ohj = oh[:, j * WIN:(j + 1) * WIN]
                        st, sp = (i == 0), (i == nb - 1)
                        if layer1:
                            nc.tensor.matmul(ps[:], feats, ohj,
                                             start=st, stop=sp)
                        else:
                            nc.tensor.matmul(ps[:], ohj, feats,
                                             start=st, stop=sp)
                        i += 1

            # ---------------- stage A: layer 1 ----------------
            with contextlib.ExitStack() as sa:
                gpool = sa.enter_context(tc.tile_pool(name="gatherA", bufs=2))
                ohpool = sa.enter_context(tc.tile_pool(name="ohA", bufs=12))
                aggpool = sa.enter_context(tc.tile_pool(name="aggT", bufs=4))
                xpool = sa.enter_context(tc.tile_pool(name="x1", bufs=4))
                psA = sa.enter_context(
                    tc.tile_pool(name="psA", bufs=4, space="PSUM"))
                psB = sa.enter_context(
                    tc.tile_pool(name="psB", bufs=2, space="PSUM"))
                psC = sa.enter_context(
                    tc.tile_pool(name="psC", bufs=2, space="PSUM"))

                cchunk = 0
                for g in range(ngroups):
                    gts = make_gts(gpool, g, "ga")
                    issue_gathers(g, gts, h128)
                    for wi in groups[g]:
                        ps = psA.tile([D, WIN], f32)
                        seg_matmul(ps, gts, wi, layer1=True)
                        aggT = aggpool.tile([D, WIN], f32)
                        nc.vector.tensor_tensor(
                            aggT[:], ps[:], hTo_sb[:, wi * WIN:(wi + 1) * WIN],
                            add_op)
                        ps2 = psB.tile([HID, WIN], f32)
                        nc.tensor.matmul(ps2[:], W1_sb[:], aggT[:],
                                         start=True, stop=True)
                        x1 = xpool.tile([HID, WIN], f32)
                        nc.scalar.activation(x1[:], ps2[:], Relu,
                                             bias=b1_sb[:, 0:1])
                        ps3 = psC.tile([WIN, D], f32)
                        nc.tensor.matmul(ps3[:], x1[:], W2_sb[:],
                                         start=True, stop=True)
                        # yown = y + b2 (self-loop + bias for layer 2)
                        nc.vector.tensor_tensor(
                            yown[:, wi * D:(wi + 1) * D], ps3[:], b2_sb[:],
                            add_op)
                        ybf = xpool.tile([WIN, ROW], f16, tag="ybf")
                        nc.scalar.copy(ybf[:, 0:D], ps3[:])
                        rows = last_rows if wi == nw - 1 else WIN
                        nc.sync.dma_start(
                            cc_in[wi * WIN: wi * WIN + rows, :],
                            ybf[:rows, :])
                    if cchunk < cc_chunks and g + 1 == gcuts[cchunk]:
                        r0, r1 = crows[cchunk]
                        nc.gpsimd.collective_compute(
                            "AllGather", mybir.AluOpType.bypass,
                            replica_groups=[list(range(CORES))],
                            ins=[cc_in.ap()[r0:r1, :].opt()],
                            outs=[ystage[cchunk].ap().opt()])
                        nst = r1 - r0
                        for k in range(CORES):
                            nc.sync.dma_start(
                                y4.ap()[k * nloc + r0:k * nloc + r1, :],
                                ystage[cchunk].ap()[k * nst:(k + 1) * nst, :])
                        cchunk += 1

            # ---------------- stage C: layer 2 ----------------
            with contextlib.ExitStack() as sc:
                gpool = sc.enter_context(tc.tile_pool(name="gatherC", bufs=2))
                ohpool = sc.enter_context(tc.tile_pool(name="ohC", bufs=12))
                spool = sc.enter_context(tc.tile_pool(name="smax", bufs=6))
                opool = sc.enter_context(tc.tile_pool(name="outp", bufs=4))
                psD = sc.enter_context(
                    tc.tile_pool(name="psD", bufs=8, space="PSUM"))

                for g in range(ngroups):
                    gts = make_gts(gpool, g, "gc")
                    issue_gathers(g, gts, y4)
                    for wi in groups[g]:
                        ps = psD.tile([WIN, D], f32)
                        seg_matmul(ps, gts, wi, layer1=False)
                        t2 = spool.tile([WIN, D], f32, tag="t2")
                        nc.vector.tensor_tensor(
                            t2[:], ps[:], yown[:, wi * D:(wi + 1) * D], add_op)
                        mx = spool.tile([WIN, 1], f32, tag="mx")
                        nc.vector.tensor_reduce(
                            mx[:], t2[:, :C], mybir.AxisListType.X,
                            mybir.AluOpType.max, negate=True)
                        e = spool.tile([WIN, C], f32, tag="e")
                        sm = spool.tile([WIN, 1], f32, tag="sm")
                        nc.scalar.activation(e[:], t2[:, :C], Exp,
                                             bias=mx[:, 0:1],
                                             accum_out=sm[:, 0:1])
                        ri = spool.tile([WIN, 1], f32, tag="ri")
                        nc.vector.reciprocal(ri[:], sm[:])
                        o = opool.tile([WIN, C], f32)
                        nc.scalar.mul(o[:], e[:], ri[:, 0:1])
                        rows = last_rows if wi == nw - 1 else WIN
                        nc.sync.dma_start(
                            outd[wi * WIN: wi * WIN + rows, :], o[:rows, :])

    nc.finalize()
    return nc


# ----------------------------------------------------------------------------
# Entry point
# ----------------------------------------------------------------------------

def _prepare_inputs(node_embeddings, adjacency_lists, W1, b1, W2, b2, rt):
    n, d = node_embeddings.shape
    nloc, nw, maxb = rt["nloc"], rt["nw"], rt["maxb"]
    nlocp = nw * WIN
    h = np.ascontiguousarray(node_embeddings, np.float32)
    h128 = np.zeros((n, ROW), np.float16)
    h128[:, :d] = h.astype(np.float16)
    W2p = np.zeros((HID, D), np.float32)
    W2p[:, :C] = W2
    b2b = np.tile(np.pad(b2.astype(np.float32), (0, D - C)), (WIN, 1))
    iotaB = np.tile(np.arange(WIN, dtype=np.float32), (WIN, maxb))
    in_maps = []
    for k in range(CORES):
        hToa = np.zeros((d, nlocp), np.float16)
        hToa[:, :nloc] = h[k * nloc:(k + 1) * nloc].T.astype(np.float16)
        in_maps.append({
            "h128": h128,
            "hTo": hToa,
            "W1": np.ascontiguousarray(W1, np.float32),
            "b1": np.ascontiguousarray(b1, np.float32).reshape(HID, 1),
            "W2p": W2p,
            "b2b": b2b,
            "idx": np.tile(rt["idx"][k].reshape(-1, 16).T, (8, 1)).copy(),
            "dstc": np.ascontiguousarray(rt["dst"][k].T).astype(np.float16),
            "iotaB": iotaB.astype(np.float16),
            "out": np.zeros((nloc, C), np.float32),
        })
    return in_maps


_CACHE = {}


def _get_program(n_nodes, rt_sig, rt):
    key = (n_nodes, rt_sig)
    if key not in _CACHE:
        _CACHE[key] = build_program(n_nodes, rt)
    return _CACHE[key]


def kernel(node_embeddings, adjacency_lists, W1, b1, W2, b2, trace=False):
    import sys
    if "/opt/trn_rl_repo" not in sys.path:
        sys.path.insert(0, "/opt/trn_rl_repo")
    from concourse import bass_utils

    n = node_embeddings.shape[0]
    src = np.asarray(adjacency_lists)[:, 0]
    dst = np.asarray(adjacency_lists)[:, 1]
    rt = route_edges(src, dst, n)
    rt_sig = (rt["tot"], rt["nseg"])
    nc = _get_program(n, rt_sig, rt)
    in_maps = _prepare_inputs(node_embeddings, adjacency_lists,
                              W1, b1, W2, b2, rt)
    res = bass_utils.run_bass_kernel_spmd(
        nc, in_maps, core_ids=list(range(CORES)), trace=trace)
    out = np.concatenate([res.results[k]["out"] for k in range(CORES)], axis=0)
    kernel.last_result = res
    return out
